# revision 42
# baseline (speedup 1.0000x reference)
"""Mixture-of-Softmaxes kernel for 8 Trainium2 NeuronCores.

Strategy: tensor-parallel over the vocab dimension (V=100000 -> 12500/core).
Each core computes all B rows for its vocab shard, HEAD-MAJOR within each
128-row block: head h's full 12500-col logit strip (fp8e4 DoubleRow
matmuls, K=256 in one PE pass) -> exp on ScalarE with fused row-sum
accumulation -> one small AllReduce per head-PAIR ([128,2] f32, 2 per
block) fired as soon as the pair's sums exist. The mixture
(out = sum_h pi_h/S_h * e_h) runs on DVE (tensor_scalar_mul at 4x +
tensor_tensor adds at 2x, in place on the ring slots), deferred one head
behind each AR so collective latency stays off the critical path; the
AR-result fetch rides the sync queue so it never head-of-line blocks the
next AR's staging on the gpsimd queue.

Key structure:
- Row sums: 5 of 7 chunks/head ride the ACT accumulator (zero extra SBUF
  traffic, one 290ns ACCUM_READ each); 2 chunks/head use the DVE in-place
  pseudo-copy accumulator, balancing ScalarE (the pacing engine, ~75%
  busy) against DVE (~60%).
- exp ring: 11 half-strip slots [128, 6356] bf16; pair ARs free the
  previous block's slots progressively, so the next block rarely starves.
- emb streamed as half-strips ([128, 2, 6356] fp8, 3-buffer rotation):
  every half's DMA starts >=1 block ahead of its first use.
- 2 warmup collectives at the very top eat the one-time ~40us+18us CC
  stream setup while the prologue (input DMAs, tanh proj, pi softmax)
  runs in parallel; blocks 0-1 defer their mixture longer since their
  ARs still pay elevated warmup latency.
- The last block fires per-head solo ARs so only the final head's
  AR + mul/add + out-DMA remain after the last exp.

Host-side prep: inputs transposed (contraction dim -> partitions); emb
pre-scaled by 16 and cast to fp8e4 (descaled for free via the exp's scale
argument); x/proj_mat/mix_mat in bf16; output bf16 -> f32 cast + vocab
concat on host.
"""

import numpy as np
import ml_dtypes

import concourse.bass as bass
import concourse.mybir as mybir
import concourse.tile as tile
from concourse import bacc
from concourse.bass_utils import run_bass_kernel_spmd
from concourse.bass_interp import get_hw_module

B, H, D, V = 1024, 4, 256, 100000
N_CORES = 8
V_S = V // N_CORES          # 12500 vocab entries per core
KT = D // 128               # 2 contraction k-tiles
N_BBLK = B // 128           # 8 b-blocks
H0W, H1W = 6144, 6356       # uneven halves of V_S (3x2048 | 3x2048+212)
SLOTW = H1W                 # ring slot width
E_SLOTS = 11                 # exp ring slots (half-strips)
# (q, offset-in-half, width): psum chunks; two [128,2048] buffers ping-pong
CHUNKS = [(0, 0, 2048), (0, 2048, 2048), (0, 4096, 2048),
          (1, 0, 2048), (1, 2048, 2048), (1, 4096, 2048), (1, 6144, 212)]
NCH = len(CHUNKS)           # chunks (= accum cols) per head

EMB_SCALE = 16.0            # host pre-scale of emb (undone in exp's scale)

F32 = mybir.dt.float32
BF16 = mybir.dt.bfloat16
FP8E4 = mybir.dt.float8e4

_RUN_KWARGS = {}  # test harness may set trace/tmpdir here
_CACHE = {}


def _build():
    nc = bacc.Bacc("TRN2", target_bir_lowering=False, debug=False,
                   num_devices=N_CORES)
    xT = nc.dram_tensor("xT", [D, B], BF16, kind="ExternalInput").ap()
    pmT = nc.dram_tensor("pmT", [D, H * D], BF16, kind="ExternalInput").ap()
    mmT = nc.dram_tensor("mmT", [D, H], BF16, kind="ExternalInput").ap()
    embT = nc.dram_tensor("embT", [128, KT * V_S], FP8E4,
                          kind="ExternalInput").ap()
    out = nc.dram_tensor("out", [B, V_S], BF16, kind="ExternalOutput").ap()

    with tile.TileContext(nc) as tc:
        _body(tc, xT, pmT, mmT, embT, out)
        tc._pool_ctx.close()

    nc.compile()
    nc.m = get_hw_module(nc.m)
    return nc


def _body(tc, xT, pmT, mmT, embT, out):
    nc = tc.nc
    Exp = mybir.ActivationFunctionType.Exp
    Tanh = mybir.ActivationFunctionType.Tanh
    add = mybir.AluOpType.add
    mult = mybir.AluOpType.mult

    import contextlib
    ctx = contextlib.ExitStack()
    tc._pool_ctx = ctx
    singles = ctx.enter_context(tc.tile_pool(name="singles", bufs=1))
    work = ctx.enter_context(tc.tile_pool(name="work", bufs=2))
    ering = ctx.enter_context(tc.tile_pool(name="ering", bufs=E_SLOTS))
    estream = ctx.enter_context(tc.tile_pool(name="estream", bufs=3))
    psum = ctx.enter_context(tc.tile_pool(name="psum", bufs=2, space="PSUM"))
    dram = ctx.enter_context(tc.tile_pool(name="dram", bufs=2, space="DRAM"))

    # ---- warm up the CC stream at the very top: the first few
    # collectives pay ~41us + ~18us of one-time setup ----
    zz = work.tile([128, 1], F32, tag="zz", name="zz")
    nc.gpsimd.memset(zz, 0.0)
    for wi in range(2):
        warm_in = dram.tile([128, 1], F32, tag=f"wrmin{wi}",
                            name=f"wrmin{wi}", bufs=1)
        warm_out = dram.tile([128, 1], F32, tag=f"wrmout{wi}",
                             name=f"wrmout{wi}", bufs=1)
        nc.gpsimd.dma_start(out=warm_in[:], in_=zz)
        nc.gpsimd.collective_compute(
            "AllReduce", add,
            replica_groups=[list(range(N_CORES))],
            ins=[warm_in.opt()], outs=[warm_out.opt()],
        )

    # ---- prologue: resident inputs ----
    sb_xT, sb_pmT, sb_mmT = [], [], []
    for k in range(KT):
        t = singles.tile([128, B], BF16, tag=f"xT{k}", name=f"xT{k}")
        nc.sync.dma_start(out=t, in_=xT[k * 128:(k + 1) * 128, :])
        sb_xT.append(t)
        t = singles.tile([128, H * D], BF16, tag=f"pmT{k}", name=f"pmT{k}")
        nc.sync.dma_start(out=t, in_=pmT[k * 128:(k + 1) * 128, :])
        sb_pmT.append(t)
        t = singles.tile([128, H], BF16, tag=f"mmT{k}", name=f"mmT{k}")
        nc.sync.dma_start(out=t, in_=mmT[k * 128:(k + 1) * 128, :])
        sb_mmT.append(t)

    ps_parity = [0]

    def next_ps():
        pstag = "psA" if ps_parity[0] % 2 == 0 else "psB"
        ps_parity[0] += 1
        return psum.tile([128, 2048], F32, tag=pstag, name=pstag, bufs=1)

    # ---- projT = tanh(proj_mat @ x.T), resident (fp8 interleaved) ----
    proj = [singles.tile([128, KT, B], FP8E4, tag=f"pj{h}", name=f"pj{h}")
            for h in range(H)]
    for h in range(H):
        for kd in range(KT):
            for bs in range(B // 512):
                ps = next_ps()
                for kc in range(KT):
                    nc.tensor.matmul(
                        ps[:, :512],
                        sb_pmT[kc][:, h * D + kd * 128:
                                    h * D + (kd + 1) * 128],
                        sb_xT[kc][:, bs * 512:(bs + 1) * 512],
                        start=(kc == 0), stop=(kc == KT - 1),
                    )
                nc.scalar.activation(
                    out=proj[h][:, kd, bs * 512:(bs + 1) * 512],
                    in_=ps[:, :512], func=Tanh)

    # ---- pi[b, h] = softmax_h(x @ mix_mat.T) per b-block ----
    # mix logits are small (|l| < ~6): exp directly, no max subtraction.
    # Emitted INSIDE block 0 (after head 0): first used by block 0's
    # pair-A drain much later, and the ~3us of ACT/DVE work lands in the
    # block-boundary stall instead of stretching the prologue.
    sb_pi = [None] * N_BBLK

    def emit_pi_all():
        for pb in range(N_BBLK):
            ps = next_ps()
            for kc in range(KT):
                nc.tensor.matmul(
                    ps[:, :H],
                    sb_xT[kc][:, pb * 128:(pb + 1) * 128],
                    sb_mmT[kc],
                    start=(kc == 0), stop=(kc == KT - 1),
                )
            e = work.tile([128, H], F32, tag="pie", name="pie")
            nc.scalar.activation(out=e, in_=ps[:, :H], func=Exp)
            s = work.tile([128, 1], F32, tag="pis", name="pis")
            nc.vector.tensor_reduce(out=s, in_=e,
                                    axis=mybir.AxisListType.X, op=add)
            rs = work.tile([128, 1], F32, tag="pirs", name="pirs")
            nc.vector.reciprocal(rs, s)
            pi = singles.tile([128, H], F32, tag=f"pi{pb}", name=f"pi{pb}")
            nc.vector.tensor_scalar_mul(pi, e, rs)
            sb_pi[pb] = pi

    # ---- main loop: head-major, head-pair AllReduces ----
    # AR j covers a pair of heads ([128,2] f32) and fires once the pair's
    # sums exist; its AR-dependent ops (w + mixture) drain one head later
    # (~14us of compute slack vs ~9us AR latency; more for the first two
    # blocks whose ARs still pay CC-stream warmup). The last block runs
    # heads [2,0,1,3] with solo ARs for heads 2 and 3, spreading the CC
    # stream and shortening the tail to one solo AR + mix + DMA.
    exp_scale = 1.0 / EMB_SCALE
    sums_t = [work.tile([128, H * NCH], F32, tag=f"sums{j}",
                        name=f"sums{j}") for j in range(2)]

    pending = []  # [(drain_key, [ops])]: deferred AR-dependent ops
    hctr = [0]    # global processed-head counter

    def drain(now=None):
        while pending and pending[0][0] <= (hctr[0] if now is None
                                            else now):
            for op in pending.pop(0)[1]:
                op()

    ar_j = [0]

    def fire_ar(i, heads, s_cols, acc, acc_head, eqs, do_dma, extra):
        """Stage + fire one AR for `heads`; queue its deferred ops."""
        j = ar_j[0]
        ar_j[0] += 1
        nh = len(heads)
        cc_in = dram.tile([128, nh], F32, tag=f"cci{j}", name=f"cci{j}",
                          bufs=1)
        cc_out = dram.tile([128, nh], F32, tag=f"cco{j}", name=f"cco{j}",
                           bufs=1)
        nc.gpsimd.dma_start(out=cc_in[:], in_=s_cols)
        nc.gpsimd.collective_compute(
            "AllReduce", add,
            replica_groups=[list(range(N_CORES))],
            ins=[cc_in.opt()], outs=[cc_out.opt()],
        )
        s_g = singles.tile([128, nh], F32, tag=f"sg{j}", name=f"sg{j}")
        rS = singles.tile([128, nh], F32, tag=f"rS{j}", name=f"rS{j}")
        w = singles.tile([128, nh], F32, tag=f"w{j}", name=f"w{j}")

        def op_w(s_g=s_g, rS=rS, w=w, cc_out=cc_out, pi=sb_pi[i],
                 h0=heads[0], nh=nh):
            # sglob fetch on the sync queue: on gpsimd it would sit
            # behind the next AR's staging DMA (which heads the queue
            # until the next head's sums exist), delaying the mixture
            nc.sync.dma_start(out=s_g, in_=cc_out[:])
            nc.vector.reciprocal(rS, s_g)
            nc.vector.tensor_mul(w, pi[:, h0:h0 + nh], rS)

        ops = [op_w]
        for q, qw in ((0, H0W), (1, H1W)):
            for k, h in enumerate(heads):
                def op_mul(eq=eqs[h][q], qw=qw, w=w, k=k):
                    nc.vector.tensor_scalar_mul(eq[:, :qw], eq[:, :qw],
                                                w[:, k:k + 1])
                ops.append(op_mul)
            for h in heads:
                if h == acc_head:
                    continue  # the accumulator head has no add
                def op_add(a=acc[q], eq=eqs[h][q], qw=qw):
                    nc.vector.tensor_tensor(out=a[:, :qw], in0=a[:, :qw],
                                            in1=eq[:, :qw], op=add)
                ops.append(op_add)
            if do_dma:
                def op_dma(a=acc[q], qw=qw, i=i, goff=q * H0W):
                    nc.sync.dma_start(
                        out=out[i * 128:(i + 1) * 128, goff:goff + qw],
                        in_=a[:, :qw])
                ops.append(op_dma)
        pending.append((hctr[0] + extra, ops))

    for i in range(N_BBLK):
        # emb halves, 3-buffer rotation: each half's DMA starts >=1 block
        # ahead of use (the buffer frees mid/end of the previous block)
        emb = []
        for q, qw in ((0, H0W), (1, H1W)):
            eh = estream.tile([128, KT, SLOTW], FP8E4, tag="emb",
                              name=f"emb{i}q{q}")
            qoff = q * H0W
            for kd in range(KT):
                nc.sync.dma_start(
                    out=eh[:, kd, :qw],
                    in_=embT[:, kd * V_S + qoff:kd * V_S + qoff + qw])
            emb.append(eh)
        last = i == N_BBLK - 1
        head_order = (0, 1, 2, 3)
        acc_head = 0
        acc = [None, None]  # first head's slots = mixture accumulator
        eqs = {}
        # blocks 0/1: ARs still pay CC warmup (~16-19us), defer longer —
        # but never past the point the next block needs the ring slots
        extra_A = 3 if i == 0 else (2 if i == 1 else 1)
        extra_B = 2 if i <= 1 else 1
        s_pair = None
        for h in head_order:
            eq = [ering.tile([128, SLOTW], BF16, tag="e",
                             name=f"e{i}_{h}q{q}") for q in range(2)]
            eqs[h] = eq
            if h == acc_head:
                acc[0], acc[1] = eq[0], eq[1]
            for cidx, (q, c0, cw) in enumerate(CHUNKS):
                if cidx == 2:
                    # mid-head drain point: a just-landed AR's ops run
                    # during this head instead of after it (used by the
                    # last block's solo ARs to keep the tail short)
                    drain(now=hctr[0] + 0.5)
                ps = next_ps()
                for ns in range((cw + 511) // 512):
                    n0 = ns * 512
                    nw = min(512, cw - n0)
                    nc.tensor.matmul(
                        ps[:, n0:n0 + nw],
                        proj[h][:, :, i * 128:(i + 1) * 128],
                        emb[q][:, :, c0 + n0:c0 + n0 + nw],
                        start=True, stop=True,
                        perf_mode=mybir.MatmulPerfMode.DoubleRow,
                    )
                dst = eq[q][:, c0:c0 + cw]
                scol = sums_t[i % 2][:, h * NCH + cidx:h * NCH + cidx + 1]
                if cidx == 3 or cw == 212:
                    # 2 chunks/head on the DVE pseudo-copy accumulator
                    # (balances ScalarE ACCUM_READ vs DVE slack)
                    nc.scalar.activation(out=dst, in_=ps[:, :cw],
                                         func=Exp, scale=exp_scale)
                    nc.vector.tensor_scalar(out=dst, in0=dst,
                                            scalar1=1.0, scalar2=None,
                                            op0=mult, op1=add,
                                            accum_out=scol)
                else:
                    # the rest ride the ACT accumulator: zero extra
                    # SBUF traffic, 300ns ACCUM_READ on ScalarE
                    nc.scalar.activation(out=dst, in_=ps[:, :cw],
                                         func=Exp, scale=exp_scale,
                                         accum_out=scol)
            solo = last  # final block: all-solo ARs shorten the tail
            if solo:
                s_head = singles.tile([128, 1], F32, tag=f"sp{i}_{h}",
                                      name=f"sp{i}_{h}")
                col = 0
            elif h % 2 == 0:
                s_pair = singles.tile([128, 2], F32, tag=f"sp{i}_{h}",
                                      name=f"sp{i}_{h}")
                s_head, col = s_pair, 0
            else:
                s_head, col = s_pair, 1
            nc.vector.tensor_reduce(
                out=s_head[:, col:col + 1],
                in_=sums_t[i % 2][:, h * NCH:(h + 1) * NCH],
                axis=mybir.AxisListType.X, op=add)
            hctr[0] += 1
            if solo:
                # solo2 drains mid-head-3 (0.5): its mul/add run during
                # head 3's exps, leaving only solo3's chain on the tail
                fire_ar(i, [h], s_head, acc, acc_head, eqs,
                        do_dma=(h == 3), extra=0.5 if h == 2 else 1)
            elif h % 2 == 1:
                fire_ar(i, [h - 1, h], s_pair, acc, acc_head, eqs,
                        do_dma=(h == 3),
                        extra=extra_A if h == 1 else extra_B)
            drain()
            if i == 0 and h == 0:
                emit_pi_all()

    # epilogue: flush the remaining ARs' ops immediately
    drain(now=10**9)


def _get_nc():
    if "nc" not in _CACHE:
        _CACHE["nc"] = _build()
    return _CACHE["nc"]


def kernel(x, proj_mat, mix_mat, emb):
    nc = _get_nc()
    bf = ml_dtypes.bfloat16
    xT = np.ascontiguousarray(x.astype(bf).T)
    pmT = np.ascontiguousarray(proj_mat.astype(bf).T)
    mmT = np.ascontiguousarray(mix_mat.astype(bf).T)
    in_maps = []
    for c in range(N_CORES):
        shard = emb[c * V_S:(c + 1) * V_S]
        # [dl, kd*V_S + v] = emb[v, kd*128+dl] * EMB_SCALE, fp8e4
        e16 = (shard.T * EMB_SCALE).astype(ml_dtypes.float8_e4m3)
        embT = np.ascontiguousarray(
            e16.reshape(KT, 128, V_S).transpose(1, 0, 2).reshape(
                128, KT * V_S))
        in_maps.append({"xT": xT, "pmT": pmT, "mmT": mmT, "embT": embT})
    res = run_bass_kernel_spmd(nc, in_maps, list(range(N_CORES)),
                               **_RUN_KWARGS)
    _CACHE["last_result"] = res
    return np.concatenate(
        [res.results[c]["out"].astype(np.float32) for c in range(N_CORES)],
        axis=1)


# revision 43
# speedup vs baseline: 1.0729x; 1.0729x over previous
"""Mixture-of-Softmaxes kernel for 8 Trainium2 NeuronCores.

Strategy: tensor-parallel over the vocab dimension (V=100000 -> 12500/core).
Each core computes all B rows for its vocab shard, HEAD-MAJOR within each
128-row block: head h's full 12500-col logit strip (fp8e4 DoubleRow
matmuls, K=256 in one PE pass) -> exp on ScalarE with fused row-sum
accumulation -> one small AllReduce per head-PAIR ([128,2] f32, 2 per
block) fired as soon as the pair's sums exist. The mixture
(out = sum_h pi_h/S_h * e_h) runs on DVE (tensor_scalar_mul at 4x +
tensor_tensor adds at 2x, in place on the ring slots), deferred one head
behind each AR so collective latency stays off the critical path; the
AR-result fetch rides the sync queue so it never head-of-line blocks the
next AR's staging on the gpsimd queue.

Key structure:
- Row sums: 5 of 7 chunks/head ride the ACT accumulator (zero extra SBUF
  traffic, one 290ns ACCUM_READ each); 2 chunks/head use the DVE in-place
  pseudo-copy accumulator, balancing ScalarE (the pacing engine, ~75%
  busy) against DVE (~60%).
- exp ring: 11 half-strip slots [128, 6356] bf16; pair ARs free the
  previous block's slots progressively, so the next block rarely starves.
- emb streamed as half-strips ([128, 2, 6356] fp8, 3-buffer rotation):
  every half's DMA starts >=1 block ahead of its first use.
- 2 warmup collectives at the very top eat the one-time ~40us+18us CC
  stream setup while the prologue (input DMAs, tanh proj, pi softmax)
  runs in parallel; blocks 0-1 defer their mixture longer since their
  ARs still pay elevated warmup latency.
- The last block fires per-head solo ARs so only the final head's
  AR + mul/add + out-DMA remain after the last exp.

Host-side prep: inputs transposed (contraction dim -> partitions); emb
pre-scaled by 16 and cast to fp8e4 (descaled for free via the exp's scale
argument); x/proj_mat/mix_mat in bf16; output bf16 -> f32 cast + vocab
concat on host.
"""

import numpy as np
import ml_dtypes

import concourse.bass as bass
import concourse.mybir as mybir
import concourse.tile as tile
from concourse import bacc
from concourse.bass_utils import run_bass_kernel_spmd
from concourse.bass_interp import get_hw_module

B, H, D, V = 1024, 4, 256, 100000
N_CORES = 8
V_S = V // N_CORES          # 12500 vocab entries per core
KT = D // 128               # 2 contraction k-tiles
N_BBLK = B // 128           # 8 b-blocks
H0W, H1W = 6144, 6356       # uneven halves of V_S (3x2048 | 3x2048+212)
SLOTW = H1W                 # ring slot width
E_SLOTS = 11                 # exp ring slots (half-strips)
# (q, offset-in-half, width): psum chunks; two [128,2048] buffers ping-pong
CHUNKS = [(0, 0, 2048), (0, 2048, 2048), (0, 4096, 2048),
          (1, 0, 2048), (1, 2048, 2048), (1, 4096, 2048), (1, 6144, 212)]
NCH = len(CHUNKS)           # chunks (= accum cols) per head

EMB_SCALE = 16.0            # host pre-scale of emb (undone in exp's scale)

F32 = mybir.dt.float32
BF16 = mybir.dt.bfloat16
FP8E4 = mybir.dt.float8e4

_RUN_KWARGS = {}  # test harness may set trace/tmpdir here
_CACHE = {}


def _build():
    nc = bacc.Bacc("TRN2", target_bir_lowering=False, debug=False,
                   num_devices=N_CORES)
    xT = nc.dram_tensor("xT", [D, B], BF16, kind="ExternalInput").ap()
    pmT = nc.dram_tensor("pmT", [D, H * D], BF16, kind="ExternalInput").ap()
    mmT = nc.dram_tensor("mmT", [D, H], BF16, kind="ExternalInput").ap()
    embT = nc.dram_tensor("embT", [128, KT * V_S], FP8E4,
                          kind="ExternalInput").ap()
    out = nc.dram_tensor("out", [B, V_S], BF16, kind="ExternalOutput").ap()

    with tile.TileContext(nc) as tc:
        _body(tc, xT, pmT, mmT, embT, out)
        tc._pool_ctx.close()

    nc.compile()
    nc.m = get_hw_module(nc.m)
    return nc


def _body(tc, xT, pmT, mmT, embT, out):
    nc = tc.nc
    Exp = mybir.ActivationFunctionType.Exp
    Tanh = mybir.ActivationFunctionType.Tanh
    add = mybir.AluOpType.add
    mult = mybir.AluOpType.mult

    import contextlib
    ctx = contextlib.ExitStack()
    tc._pool_ctx = ctx
    singles = ctx.enter_context(tc.tile_pool(name="singles", bufs=1))
    work = ctx.enter_context(tc.tile_pool(name="work", bufs=2))
    ering = ctx.enter_context(tc.tile_pool(name="ering", bufs=E_SLOTS))
    estream = ctx.enter_context(tc.tile_pool(name="estream", bufs=3))
    psum = ctx.enter_context(tc.tile_pool(name="psum", bufs=2, space="PSUM"))
    dram = ctx.enter_context(tc.tile_pool(name="dram", bufs=2, space="DRAM"))

    # ---- warm up the CC stream at the very top: the first few
    # collectives pay ~41us + ~18us of one-time setup ----
    zz = work.tile([128, 1], F32, tag="zz", name="zz")
    nc.gpsimd.memset(zz, 0.0)
    for wi in range(2):
        warm_in = dram.tile([128, 1], F32, tag=f"wrmin{wi}",
                            name=f"wrmin{wi}", bufs=1)
        warm_out = dram.tile([128, 1], F32, tag=f"wrmout{wi}",
                             name=f"wrmout{wi}", bufs=1)
        nc.gpsimd.dma_start(out=warm_in[:], in_=zz)
        nc.gpsimd.collective_compute(
            "AllReduce", add,
            replica_groups=[list(range(N_CORES))],
            ins=[warm_in.opt()], outs=[warm_out.opt()],
        )

    # ---- prologue: resident inputs ----
    sb_xT, sb_pmT, sb_mmT = [], [], []
    for k in range(KT):
        t = singles.tile([128, B], BF16, tag=f"xT{k}", name=f"xT{k}")
        nc.sync.dma_start(out=t, in_=xT[k * 128:(k + 1) * 128, :])
        sb_xT.append(t)
        t = singles.tile([128, H * D], BF16, tag=f"pmT{k}", name=f"pmT{k}")
        nc.sync.dma_start(out=t, in_=pmT[k * 128:(k + 1) * 128, :])
        sb_pmT.append(t)
        t = singles.tile([128, H], BF16, tag=f"mmT{k}", name=f"mmT{k}")
        nc.sync.dma_start(out=t, in_=mmT[k * 128:(k + 1) * 128, :])
        sb_mmT.append(t)

    ps_parity = [0]

    def next_ps():
        pstag = "psA" if ps_parity[0] % 2 == 0 else "psB"
        ps_parity[0] += 1
        return psum.tile([128, 2048], F32, tag=pstag, name=pstag, bufs=1)

    # ---- projT = tanh(proj_mat @ x.T), resident (fp8 interleaved) ----
    proj = [singles.tile([128, KT, B], FP8E4, tag=f"pj{h}", name=f"pj{h}")
            for h in range(H)]
    for h in range(H):
        for kd in range(KT):
            for bs in range(B // 512):
                ps = next_ps()
                for kc in range(KT):
                    nc.tensor.matmul(
                        ps[:, :512],
                        sb_pmT[kc][:, h * D + kd * 128:
                                    h * D + (kd + 1) * 128],
                        sb_xT[kc][:, bs * 512:(bs + 1) * 512],
                        start=(kc == 0), stop=(kc == KT - 1),
                    )
                nc.scalar.activation(
                    out=proj[h][:, kd, bs * 512:(bs + 1) * 512],
                    in_=ps[:, :512], func=Tanh)

    # ---- pi[b, h] = softmax_h(x @ mix_mat.T) per b-block ----
    # mix logits are small (|l| < ~6): exp directly, no max subtraction
    sb_pi = [None] * N_BBLK
    if True:
        for pb in range(N_BBLK):
            ps = next_ps()
            for kc in range(KT):
                nc.tensor.matmul(
                    ps[:, :H],
                    sb_xT[kc][:, pb * 128:(pb + 1) * 128],
                    sb_mmT[kc],
                    start=(kc == 0), stop=(kc == KT - 1),
                )
            e = work.tile([128, H], F32, tag="pie", name="pie")
            nc.scalar.activation(out=e, in_=ps[:, :H], func=Exp)
            s = work.tile([128, 1], F32, tag="pis", name="pis")
            nc.vector.tensor_reduce(out=s, in_=e,
                                    axis=mybir.AxisListType.X, op=add)
            rs = work.tile([128, 1], F32, tag="pirs", name="pirs")
            nc.vector.reciprocal(rs, s)
            pi = singles.tile([128, H], F32, tag=f"pi{pb}", name=f"pi{pb}")
            nc.vector.tensor_scalar_mul(pi, e, rs)
            sb_pi[pb] = pi

    # ---- main loop: head-major, head-pair AllReduces ----
    # AR j covers a pair of heads ([128,2] f32) and fires once the pair's
    # sums exist; its AR-dependent ops (w + mixture) drain one head later
    # (~14us of compute slack vs ~9us AR latency; more for the first two
    # blocks whose ARs still pay CC-stream warmup). The last block runs
    # heads [2,0,1,3] with solo ARs for heads 2 and 3, spreading the CC
    # stream and shortening the tail to one solo AR + mix + DMA.
    exp_scale = 1.0 / EMB_SCALE
    sums_t = [work.tile([128, H * NCH], F32, tag=f"sums{j}",
                        name=f"sums{j}") for j in range(2)]

    pending = []  # [(drain_key, [ops])]: deferred AR-dependent ops
    hctr = [0]    # global processed-head counter

    def drain(now=None):
        while pending and pending[0][0] <= (hctr[0] if now is None
                                            else now):
            for op in pending.pop(0)[1]:
                op()

    ar_j = [0]

    def fire_ar(i, heads, s_cols, acc, acc_head, eqs, do_dma, extra):
        """Stage + fire one AR for `heads`; queue its deferred ops."""
        j = ar_j[0]
        ar_j[0] += 1
        nh = len(heads)
        cc_in = dram.tile([128, nh], F32, tag=f"cci{j}", name=f"cci{j}",
                          bufs=1)
        cc_out = dram.tile([128, nh], F32, tag=f"cco{j}", name=f"cco{j}",
                           bufs=1)
        nc.gpsimd.dma_start(out=cc_in[:], in_=s_cols)
        nc.gpsimd.collective_compute(
            "AllReduce", add,
            replica_groups=[list(range(N_CORES))],
            ins=[cc_in.opt()], outs=[cc_out.opt()],
        )
        s_g = singles.tile([128, nh], F32, tag=f"sg{j}", name=f"sg{j}")
        rS = singles.tile([128, nh], F32, tag=f"rS{j}", name=f"rS{j}")
        w = singles.tile([128, nh], F32, tag=f"w{j}", name=f"w{j}")

        def op_w(s_g=s_g, rS=rS, w=w, cc_out=cc_out, pi=sb_pi[i],
                 h0=heads[0], nh=nh):
            # sglob fetch on the sync queue: on gpsimd it would sit
            # behind the next AR's staging DMA (which heads the queue
            # until the next head's sums exist), delaying the mixture
            nc.sync.dma_start(out=s_g, in_=cc_out[:])
            nc.vector.reciprocal(rS, s_g)
            nc.vector.tensor_mul(w, pi[:, h0:h0 + nh], rS)

        ops = [op_w]
        for q, qw in ((0, H0W), (1, H1W)):
            for k, h in enumerate(heads):
                def op_mul(eq=eqs[h][q], qw=qw, w=w, k=k):
                    nc.vector.tensor_scalar_mul(eq[:, :qw], eq[:, :qw],
                                                w[:, k:k + 1])
                ops.append(op_mul)
            for h in heads:
                if h == acc_head:
                    continue  # the accumulator head has no add
                def op_add(a=acc[q], eq=eqs[h][q], qw=qw):
                    nc.vector.tensor_tensor(out=a[:, :qw], in0=a[:, :qw],
                                            in1=eq[:, :qw], op=add)
                ops.append(op_add)
            if do_dma:
                def op_dma(a=acc[q], qw=qw, i=i, goff=q * H0W):
                    nc.sync.dma_start(
                        out=out[i * 128:(i + 1) * 128, goff:goff + qw],
                        in_=a[:, :qw])
                ops.append(op_dma)
        pending.append((hctr[0] + extra, ops))

    for i in range(N_BBLK):
        # emb halves, 3-buffer rotation: each half's DMA starts >=1 block
        # ahead of use (the buffer frees mid/end of the previous block)
        emb = []
        for q, qw in ((0, H0W), (1, H1W)):
            eh = estream.tile([128, KT, SLOTW], FP8E4, tag="emb",
                              name=f"emb{i}q{q}")
            qoff = q * H0W
            for kd in range(KT):
                nc.sync.dma_start(
                    out=eh[:, kd, :qw],
                    in_=embT[:, kd * V_S + qoff:kd * V_S + qoff + qw])
            emb.append(eh)
        last = i == N_BBLK - 1
        head_order = (0, 1, 2, 3)
        acc_head = 0
        acc = [None, None]  # first head's slots = mixture accumulator
        eqs = {}
        # blocks 0/1: ARs still pay CC warmup (~16-19us), defer longer —
        # but never past the point the next block needs the ring slots
        extra_A = 3 if i == 0 else (2 if i == 1 else 1)
        extra_B = 2 if i <= 1 else 1
        s_pair = None
        for h in head_order:
            eq = [ering.tile([128, SLOTW], BF16, tag="e",
                             name=f"e{i}_{h}q{q}") for q in range(2)]
            eqs[h] = eq
            if h == acc_head:
                acc[0], acc[1] = eq[0], eq[1]
            for cidx, (q, c0, cw) in enumerate(CHUNKS):
                ps = next_ps()
                for ns in range((cw + 511) // 512):
                    n0 = ns * 512
                    nw = min(512, cw - n0)
                    nc.tensor.matmul(
                        ps[:, n0:n0 + nw],
                        proj[h][:, :, i * 128:(i + 1) * 128],
                        emb[q][:, :, c0 + n0:c0 + n0 + nw],
                        start=True, stop=True,
                        perf_mode=mybir.MatmulPerfMode.DoubleRow,
                    )
                dst = eq[q][:, c0:c0 + cw]
                scol = sums_t[i % 2][:, h * NCH + cidx:h * NCH + cidx + 1]
                if cidx == 3 or cw == 212:
                    # 2 chunks/head on the DVE pseudo-copy accumulator
                    # (balances ScalarE ACCUM_READ vs DVE slack)
                    nc.scalar.activation(out=dst, in_=ps[:, :cw],
                                         func=Exp, scale=exp_scale)
                    nc.vector.tensor_scalar(out=dst, in0=dst,
                                            scalar1=1.0, scalar2=None,
                                            op0=mult, op1=add,
                                            accum_out=scol)
                else:
                    # the rest ride the ACT accumulator: zero extra
                    # SBUF traffic, 300ns ACCUM_READ on ScalarE
                    nc.scalar.activation(out=dst, in_=ps[:, :cw],
                                         func=Exp, scale=exp_scale,
                                         accum_out=scol)
            solo = last  # final block: all-solo ARs shorten the tail
            if solo:
                s_head = singles.tile([128, 1], F32, tag=f"sp{i}_{h}",
                                      name=f"sp{i}_{h}")
                col = 0
            elif h % 2 == 0:
                s_pair = singles.tile([128, 2], F32, tag=f"sp{i}_{h}",
                                      name=f"sp{i}_{h}")
                s_head, col = s_pair, 0
            else:
                s_head, col = s_pair, 1
            nc.vector.tensor_reduce(
                out=s_head[:, col:col + 1],
                in_=sums_t[i % 2][:, h * NCH:(h + 1) * NCH],
                axis=mybir.AxisListType.X, op=add)
            hctr[0] += 1
            if solo:
                fire_ar(i, [h], s_head, acc, acc_head, eqs,
                        do_dma=(h == 3), extra=1)
            elif h % 2 == 1:
                fire_ar(i, [h - 1, h], s_pair, acc, acc_head, eqs,
                        do_dma=(h == 3),
                        extra=extra_A if h == 1 else extra_B)
            drain()

    # epilogue: flush the remaining ARs' ops immediately
    drain(now=10**9)


def _get_nc():
    if "nc" not in _CACHE:
        _CACHE["nc"] = _build()
    return _CACHE["nc"]


def kernel(x, proj_mat, mix_mat, emb):
    nc = _get_nc()
    bf = ml_dtypes.bfloat16
    xT = np.ascontiguousarray(x.astype(bf).T)
    pmT = np.ascontiguousarray(proj_mat.astype(bf).T)
    mmT = np.ascontiguousarray(mix_mat.astype(bf).T)
    in_maps = []
    for c in range(N_CORES):
        shard = emb[c * V_S:(c + 1) * V_S]
        # [dl, kd*V_S + v] = emb[v, kd*128+dl] * EMB_SCALE, fp8e4
        e16 = (shard.T * EMB_SCALE).astype(ml_dtypes.float8_e4m3)
        embT = np.ascontiguousarray(
            e16.reshape(KT, 128, V_S).transpose(1, 0, 2).reshape(
                128, KT * V_S))
        in_maps.append({"xT": xT, "pmT": pmT, "mmT": mmT, "embT": embT})
    res = run_bass_kernel_spmd(nc, in_maps, list(range(N_CORES)),
                               **_RUN_KWARGS)
    _CACHE["last_result"] = res
    return np.concatenate(
        [res.results[c]["out"].astype(np.float32) for c in range(N_CORES)],
        axis=1)


# revision 45
# speedup vs baseline: 1.0993x; 1.0246x over previous
"""Mixture-of-Softmaxes kernel for 8 Trainium2 NeuronCores.

Strategy: tensor-parallel over the vocab dimension (V=100000 -> 12500/core).
Each core computes all B rows for its vocab shard, HEAD-MAJOR within each
128-row block: head h's full 12500-col logit strip (fp8e4 DoubleRow
matmuls, K=256 in one PE pass) -> exp on ScalarE with fused row-sum
accumulation -> one small AllReduce per head-PAIR ([128,2] f32, 2 per
block) fired as soon as the pair's sums exist. The mixture
(out = sum_h pi_h/S_h * e_h) runs on DVE (tensor_scalar_mul at 4x +
tensor_tensor adds at 2x, in place on the ring slots), deferred one head
behind each AR so collective latency stays off the critical path; the
AR-result fetch rides the sync queue so it never head-of-line blocks the
next AR's staging on the gpsimd queue.

Key structure:
- Row sums: 5 of 7 chunks/head ride the ACT accumulator (zero extra SBUF
  traffic, one 290ns ACCUM_READ each); 2 chunks/head use the DVE in-place
  pseudo-copy accumulator, balancing ScalarE (the pacing engine, ~75%
  busy) against DVE (~60%).
- exp ring: 11 half-strip slots [128, 6356] bf16; pair ARs free the
  previous block's slots progressively, so the next block rarely starves.
- emb streamed as half-strips ([128, 2, 6356] fp8, 3-buffer rotation):
  every half's DMA starts >=1 block ahead of its first use.
- 2 warmup collectives at the very top eat the one-time ~40us+18us CC
  stream setup while the prologue (input DMAs, tanh proj, pi softmax)
  runs in parallel; blocks 0-1 defer their mixture longer since their
  ARs still pay elevated warmup latency.
- The last block fires per-head solo ARs so only the final head's
  AR + mul/add + out-DMA remain after the last exp.

Host-side prep: inputs transposed (contraction dim -> partitions); emb
pre-scaled by 16 and cast to fp8e4 (descaled for free via the exp's scale
argument); x/proj_mat/mix_mat in bf16; output bf16 -> f32 cast + vocab
concat on host.
"""

import numpy as np
import ml_dtypes

import concourse.bass as bass
import concourse.mybir as mybir
import concourse.tile as tile
from concourse import bacc
from concourse.bass_utils import run_bass_kernel_spmd
from concourse.bass_interp import get_hw_module

B, H, D, V = 1024, 4, 256, 100000
N_CORES = 8
V_S = V // N_CORES          # 12500 vocab entries per core
KT = D // 128               # 2 contraction k-tiles
N_BBLK = B // 128           # 8 b-blocks
H0W, H1W = 6144, 6356       # uneven halves of V_S (3x2048 | 3x2048+212)
SLOTW = H1W                 # ring slot width
E_SLOTS = 11                 # exp ring slots (half-strips)
# (q, offset-in-half, width): psum chunks; two [128,2048] buffers ping-pong
CHUNKS = [(0, 0, 2048), (0, 2048, 2048), (0, 4096, 2048),
          (1, 0, 2048), (1, 2048, 2048), (1, 4096, 2048), (1, 6144, 212)]
NCH = len(CHUNKS)           # chunks (= accum cols) per head

EMB_SCALE = 16.0            # host pre-scale of emb (undone in exp's scale)

F32 = mybir.dt.float32
BF16 = mybir.dt.bfloat16
FP8E4 = mybir.dt.float8e4

_RUN_KWARGS = {}  # test harness may set trace/tmpdir here
_CACHE = {}


def _build():
    nc = bacc.Bacc("TRN2", target_bir_lowering=False, debug=False,
                   num_devices=N_CORES)
    xT = nc.dram_tensor("xT", [D, B], BF16, kind="ExternalInput").ap()
    pmT = nc.dram_tensor("pmT", [D, H * D], BF16, kind="ExternalInput").ap()
    mmT = nc.dram_tensor("mmT", [D, H], BF16, kind="ExternalInput").ap()
    embT = nc.dram_tensor("embT", [128, KT * V_S], FP8E4,
                          kind="ExternalInput").ap()
    out = nc.dram_tensor("out", [B, V_S], BF16, kind="ExternalOutput").ap()

    with tile.TileContext(nc) as tc:
        _body(tc, xT, pmT, mmT, embT, out)
        tc._pool_ctx.close()

    nc.compile()
    nc.m = get_hw_module(nc.m)
    return nc


def _body(tc, xT, pmT, mmT, embT, out):
    nc = tc.nc
    Exp = mybir.ActivationFunctionType.Exp
    Tanh = mybir.ActivationFunctionType.Tanh
    add = mybir.AluOpType.add
    mult = mybir.AluOpType.mult

    import contextlib
    ctx = contextlib.ExitStack()
    tc._pool_ctx = ctx
    singles = ctx.enter_context(tc.tile_pool(name="singles", bufs=1))
    work = ctx.enter_context(tc.tile_pool(name="work", bufs=2))
    ering = ctx.enter_context(tc.tile_pool(name="ering", bufs=E_SLOTS))
    estream = ctx.enter_context(tc.tile_pool(name="estream", bufs=3))
    psum = ctx.enter_context(tc.tile_pool(name="psum", bufs=2, space="PSUM"))
    dram = ctx.enter_context(tc.tile_pool(name="dram", bufs=2, space="DRAM"))

    # ---- warm up the CC stream at the very top: the first few
    # collectives pay ~41us + ~18us of one-time setup ----
    zz = work.tile([128, 1], F32, tag="zz", name="zz")
    nc.gpsimd.memset(zz, 0.0)
    for wi in range(2):
        warm_in = dram.tile([128, 1], F32, tag=f"wrmin{wi}",
                            name=f"wrmin{wi}", bufs=1)
        warm_out = dram.tile([128, 1], F32, tag=f"wrmout{wi}",
                             name=f"wrmout{wi}", bufs=1)
        nc.gpsimd.dma_start(out=warm_in[:], in_=zz)
        nc.gpsimd.collective_compute(
            "AllReduce", add,
            replica_groups=[list(range(N_CORES))],
            ins=[warm_in.opt()], outs=[warm_out.opt()],
        )

    # ---- prologue: resident inputs ----
    sb_xT, sb_pmT, sb_mmT = [], [], []
    for k in range(KT):
        t = singles.tile([128, B], BF16, tag=f"xT{k}", name=f"xT{k}")
        nc.sync.dma_start(out=t, in_=xT[k * 128:(k + 1) * 128, :])
        sb_xT.append(t)
        t = singles.tile([128, H * D], BF16, tag=f"pmT{k}", name=f"pmT{k}")
        nc.sync.dma_start(out=t, in_=pmT[k * 128:(k + 1) * 128, :])
        sb_pmT.append(t)
        t = singles.tile([128, H], BF16, tag=f"mmT{k}", name=f"mmT{k}")
        nc.sync.dma_start(out=t, in_=mmT[k * 128:(k + 1) * 128, :])
        sb_mmT.append(t)

    ps_parity = [0]

    def next_ps():
        pstag = "psA" if ps_parity[0] % 2 == 0 else "psB"
        ps_parity[0] += 1
        return psum.tile([128, 2048], F32, tag=pstag, name=pstag, bufs=1)

    # ---- projT = tanh(proj_mat @ x.T), resident (fp8 interleaved) ----
    proj = [singles.tile([128, KT, B], FP8E4, tag=f"pj{h}", name=f"pj{h}")
            for h in range(H)]
    for h in range(H):
        for kd in range(KT):
            for bs in range(B // 512):
                ps = next_ps()
                for kc in range(KT):
                    nc.tensor.matmul(
                        ps[:, :512],
                        sb_pmT[kc][:, h * D + kd * 128:
                                    h * D + (kd + 1) * 128],
                        sb_xT[kc][:, bs * 512:(bs + 1) * 512],
                        start=(kc == 0), stop=(kc == KT - 1),
                    )
                nc.scalar.activation(
                    out=proj[h][:, kd, bs * 512:(bs + 1) * 512],
                    in_=ps[:, :512], func=Tanh)

    # ---- pi[b, h] = softmax_h(x @ mix_mat.T) per b-block ----
    # mix logits are small (|l| < ~6): exp directly, no max subtraction
    sb_pi = [None] * N_BBLK
    if True:
        for pb in range(N_BBLK):
            ps = next_ps()
            for kc in range(KT):
                nc.tensor.matmul(
                    ps[:, :H],
                    sb_xT[kc][:, pb * 128:(pb + 1) * 128],
                    sb_mmT[kc],
                    start=(kc == 0), stop=(kc == KT - 1),
                )
            e = work.tile([128, H], F32, tag="pie", name="pie")
            nc.scalar.activation(out=e, in_=ps[:, :H], func=Exp)
            s = work.tile([128, 1], F32, tag="pis", name="pis")
            nc.vector.tensor_reduce(out=s, in_=e,
                                    axis=mybir.AxisListType.X, op=add)
            rs = work.tile([128, 1], F32, tag="pirs", name="pirs")
            nc.vector.reciprocal(rs, s)
            pi = singles.tile([128, H], F32, tag=f"pi{pb}", name=f"pi{pb}")
            nc.vector.tensor_scalar_mul(pi, e, rs)
            sb_pi[pb] = pi

    # ---- main loop: head-major, head-pair AllReduces ----
    # AR j covers a pair of heads ([128,2] f32) and fires once the pair's
    # sums exist; its AR-dependent ops (w + mixture) drain one head later
    # (~14us of compute slack vs ~9us AR latency; more for the first two
    # blocks whose ARs still pay CC-stream warmup). The last block runs
    # heads [2,0,1,3] with solo ARs for heads 2 and 3, spreading the CC
    # stream and shortening the tail to one solo AR + mix + DMA.
    exp_scale = 1.0 / EMB_SCALE
    sums_t = [work.tile([128, H * NCH], F32, tag=f"sums{j}",
                        name=f"sums{j}") for j in range(2)]

    pending = []  # [(drain_key, [ops])]: deferred AR-dependent ops
    hctr = [0]    # global processed-head counter

    def drain(now=None):
        while pending and pending[0][0] <= (hctr[0] if now is None
                                            else now):
            for op in pending.pop(0)[1]:
                op()

    ar_j = [0]

    def fire_ar(i, heads, s_cols, acc, acc_head, eqs, do_dma, extra):
        """Stage + fire one AR for `heads`; queue its deferred ops."""
        j = ar_j[0]
        ar_j[0] += 1
        nh = len(heads)
        cc_in = dram.tile([128, nh], F32, tag=f"cci{j}", name=f"cci{j}",
                          bufs=1)
        cc_out = dram.tile([128, nh], F32, tag=f"cco{j}", name=f"cco{j}",
                           bufs=1)
        nc.gpsimd.dma_start(out=cc_in[:], in_=s_cols)
        nc.gpsimd.collective_compute(
            "AllReduce", add,
            replica_groups=[list(range(N_CORES))],
            ins=[cc_in.opt()], outs=[cc_out.opt()],
        )
        s_g = singles.tile([128, nh], F32, tag=f"sg{j}", name=f"sg{j}")
        rS = singles.tile([128, nh], F32, tag=f"rS{j}", name=f"rS{j}")
        w = singles.tile([128, nh], F32, tag=f"w{j}", name=f"w{j}")

        def op_w(s_g=s_g, rS=rS, w=w, cc_out=cc_out, pi=sb_pi[i],
                 h0=heads[0], nh=nh):
            # sglob fetch on the sync queue: on gpsimd it would sit
            # behind the next AR's staging DMA (which heads the queue
            # until the next head's sums exist), delaying the mixture
            nc.sync.dma_start(out=s_g, in_=cc_out[:])
            nc.vector.reciprocal(rS, s_g)
            nc.vector.tensor_mul(w, pi[:, h0:h0 + nh], rS)

        ops = [op_w]
        for q, qw in ((0, H0W), (1, H1W)):
            for k, h in enumerate(heads):
                def op_mul(eq=eqs[h][q], qw=qw, w=w, k=k):
                    nc.vector.tensor_scalar_mul(eq[:, :qw], eq[:, :qw],
                                                w[:, k:k + 1])
                ops.append(op_mul)
            for h in heads:
                if h == acc_head:
                    continue  # the accumulator head has no add
                def op_add(a=acc[q], eq=eqs[h][q], qw=qw):
                    nc.vector.tensor_tensor(out=a[:, :qw], in0=a[:, :qw],
                                            in1=eq[:, :qw], op=add)
                ops.append(op_add)
            if do_dma:
                def op_dma(a=acc[q], qw=qw, i=i, goff=q * H0W):
                    nc.sync.dma_start(
                        out=out[i * 128:(i + 1) * 128, goff:goff + qw],
                        in_=a[:, :qw])
                ops.append(op_dma)
        pending.append((hctr[0] + extra, ops))

    for i in range(N_BBLK):
        # emb halves, 3-buffer rotation: each half's DMA starts >=1 block
        # ahead of use (the buffer frees mid/end of the previous block)
        emb = []
        for q, qw in ((0, H0W), (1, H1W)):
            eh = estream.tile([128, KT, SLOTW], FP8E4, tag="emb",
                              name=f"emb{i}q{q}")
            qoff = q * H0W
            for kd in range(KT):
                nc.sync.dma_start(
                    out=eh[:, kd, :qw],
                    in_=embT[:, kd * V_S + qoff:kd * V_S + qoff + qw])
            emb.append(eh)
        last = i == N_BBLK - 1
        head_order = (0, 1, 2, 3)
        acc_head = 0
        acc = [None, None]  # first head's slots = mixture accumulator
        eqs = {}
        # blocks 0/1: ARs still pay CC warmup (~16-19us), defer longer —
        # but never past the point the next block needs the ring slots
        extra_A = 3 if i == 0 else (2 if i == 1 else 1)
        extra_B = 2 if i <= 1 else 1
        s_pair = None
        for h in head_order:
            eq = [ering.tile([128, SLOTW], BF16, tag="e",
                             name=f"e{i}_{h}q{q}") for q in range(2)]
            eqs[h] = eq
            if h == acc_head:
                acc[0], acc[1] = eq[0], eq[1]
            for cidx, (q, c0, cw) in enumerate(CHUNKS):
                ps = next_ps()
                for ns in range((cw + 511) // 512):
                    n0 = ns * 512
                    nw = min(512, cw - n0)
                    nc.tensor.matmul(
                        ps[:, n0:n0 + nw],
                        proj[h][:, :, i * 128:(i + 1) * 128],
                        emb[q][:, :, c0 + n0:c0 + n0 + nw],
                        start=True, stop=True,
                        perf_mode=mybir.MatmulPerfMode.DoubleRow,
                    )
                dst = eq[q][:, c0:c0 + cw]
                scol = sums_t[i % 2][:, h * NCH + cidx:h * NCH + cidx + 1]
                if cidx == 3 or cw == 212:
                    # 2 chunks/head on the DVE pseudo-copy accumulator
                    # (balances ScalarE ACCUM_READ vs DVE slack)
                    nc.scalar.activation(out=dst, in_=ps[:, :cw],
                                         func=Exp, scale=exp_scale)
                    nc.vector.tensor_scalar(out=dst, in0=dst,
                                            scalar1=1.0, scalar2=None,
                                            op0=mult, op1=add,
                                            accum_out=scol)
                else:
                    # the rest ride the ACT accumulator: zero extra
                    # SBUF traffic, 300ns ACCUM_READ on ScalarE
                    nc.scalar.activation(out=dst, in_=ps[:, :cw],
                                         func=Exp, scale=exp_scale,
                                         accum_out=scol)
            solo = last  # final block: all-solo ARs shorten the tail
            if solo:
                s_head = singles.tile([128, 1], F32, tag=f"sp{i}_{h}",
                                      name=f"sp{i}_{h}")
                col = 0
            elif h % 2 == 0:
                s_pair = singles.tile([128, 2], F32, tag=f"sp{i}_{h}",
                                      name=f"sp{i}_{h}")
                s_head, col = s_pair, 0
            else:
                s_head, col = s_pair, 1
            nc.vector.tensor_reduce(
                out=s_head[:, col:col + 1],
                in_=sums_t[i % 2][:, h * NCH:(h + 1) * NCH],
                axis=mybir.AxisListType.X, op=add)
            hctr[0] += 1
            if solo:
                fire_ar(i, [h], s_head, acc, acc_head, eqs,
                        do_dma=(h == 3), extra=1)
            elif h % 2 == 1:
                fire_ar(i, [h - 1, h], s_pair, acc, acc_head, eqs,
                        do_dma=(h == 3),
                        extra=extra_A if h == 1 else extra_B)
            drain()

    # epilogue: flush the remaining ARs' ops immediately
    drain(now=10**9)


def _get_nc():
    if "nc" not in _CACHE:
        _CACHE["nc"] = _build()
    return _CACHE["nc"]


def kernel(x, proj_mat, mix_mat, emb):
    nc = _get_nc()
    bf = ml_dtypes.bfloat16
    xT = np.ascontiguousarray(x.astype(bf).T)
    pmT = np.ascontiguousarray(proj_mat.astype(bf).T)
    mmT = np.ascontiguousarray(mix_mat.astype(bf).T)
    in_maps = []
    for c in range(N_CORES):
        shard = emb[c * V_S:(c + 1) * V_S]
        # [dl, kd*V_S + v] = emb[v, kd*128+dl] * EMB_SCALE, fp8e4
        e16 = (shard.T * EMB_SCALE).astype(ml_dtypes.float8_e4m3)
        embT = np.ascontiguousarray(
            e16.reshape(KT, 128, V_S).transpose(1, 0, 2).reshape(
                128, KT * V_S))
        in_maps.append({"xT": xT, "pmT": pmT, "mmT": mmT, "embT": embT})
    res = run_bass_kernel_spmd(nc, in_maps, list(range(N_CORES)),
                               **_RUN_KWARGS)
    _CACHE["last_result"] = res
    return np.concatenate(
        [res.results[c]["out"].astype(np.float32) for c in range(N_CORES)],
        axis=1)
